# revision 1
# baseline (speedup 1.0000x reference)
"""Trainium2 Bass kernel for nn_EulerCausalAttention_75892072121064. v4.

Sharding: batch*heads across 8 cores (core c -> batch c//4, heads 4*(c%4)..+4).
Each core computes transposed-layout causal attention for its (b, 4-head)
slice plus the out-proj partial, writing outT (D, S). Host sums the 4
per-batch partials and transposes back.

v4: every engine queue is in-order, so emission order is scheduling.
- Exp kb-major, 1024-wide query windows, one causal-trimmed exp per
  (head, kb); kb loop software-pipelined (AV for kb-1 emitted after the
  score matmuls for kb).
- Head blocks emitted t0-heads first: [qh0 h0,h1], [qh1 h0,h1], then the
  t1 feature Sins, [qh0 h2,h3], [qh1 h2,h3] - so the first exp runs right
  after the 4 t0 Sins instead of behind all 16.
- Sin features packed [128, 2048] (cos;sin assembled by DVE cross-partition
  copies directly into qt/kt, Sin applied in place): 8 Sin calls total.
- Causal diag mask via identity x (-1e5 upper-tri) matmul accumulated into
  the scores PSUM after the chunk matmul (safe under bank- or region-clear).
- Normalize per head: denom row -> broadcast -> reciprocal -> one
  cross-partition TT mult straight into the head-pair tile.
- Out-proj per query-half; outproj(0) interleaved od-by-od into a later
  head block; outproj(1) drains PSUM on the scalar engine (idle at tail).
"""
import sys

import numpy as np

sys.path.insert(0, "/opt/trn_rl_repo")

from concourse import bacc, mybir  # noqa: E402
import concourse.tile as tile  # noqa: E402
from concourse.bass_utils import run_bass_kernel_spmd  # noqa: E402

B, S, D, H, DH = 2, 2048, 1024, 16, 64
LUT = 4096
TWO_PI = 2.0 * np.pi
SCALE = float(np.sqrt(np.float32(2.0 * DH)))  # sqrt(128) as f32
NCORES = 8
HPC = 4            # heads per core
CW = HPC * DH      # 256 cols per core
QW = 1024          # query window (half of S)
C_LUT = float(np.float32(TWO_PI / LUT))
MAGIC = float(np.float32(12582912.0))  # 1.5*2^23: x+M-M == rne-round(x)
NS = S // 128      # seq tiles

F32 = mybir.dt.float32
F32R = mybir.dt.float32r
F16 = mybir.dt.float16  # holds LUT indices (|k| <= 2048) exactly
AF = mybir.ActivationFunctionType
ALU = mybir.AluOpType

_CACHE = {}


def _build_nc():
    nc = bacc.Bacc("TRN2", debug=False, num_devices=NCORES)

    xbT = nc.dram_tensor("xbT", [D, S], F32R, kind="ExternalInput")
    vwT = nc.dram_tensor("vwT", [D, CW], F32R, kind="ExternalInput")
    owT = nc.dram_tensor("owT", [CW, D], F32R, kind="ExternalInput")
    invq = nc.dram_tensor("invq", [128, 2], F32, kind="ExternalInput")
    bq = nc.dram_tensor("bq", [128, 2], F32, kind="ExternalInput")
    invk = nc.dram_tensor("invk", [128, 2], F32, kind="ExternalInput")
    bk = nc.dram_tensor("bk", [128, 2], F32, kind="ExternalInput")
    ngt = nc.dram_tensor("ngt", [128, 128], F32R, kind="ExternalInput")
    idt = nc.dram_tensor("idt", [128, 128], F32R, kind="ExternalInput")
    outT = nc.dram_tensor("outT", [D, S], F32, kind="ExternalOutput")

    inv_scale = float(1.0 / np.float32(SCALE))

    with tile.TileContext(nc) as tc:
        with (
            tc.tile_pool(name="persist", bufs=1) as pp,
            tc.tile_pool(name="qkt", bufs=1) as qkp,
            tc.tile_pool(name="vtiles", bufs=1) as vp,
            tc.tile_pool(name="argp", bufs=1) as agp,
        ):
            invq_sb = pp.tile([128, 2], F32, tag="invq")
            nc.sync.dma_start(invq_sb[:], invq[:])
            bq_sb = pp.tile([128, 2], F32, tag="bq")
            nc.sync.dma_start(bq_sb[:], bq[:])
            invk_sb = pp.tile([128, 2], F32, tag="invk")
            nc.sync.dma_start(invk_sb[:], invk[:])
            bk_sb = pp.tile([128, 2], F32, tag="bk")
            nc.sync.dma_start(bk_sb[:], bk[:])
            ngt_sb = pp.tile([128, 128], F32R, tag="ngt")
            nc.sync.dma_start(ngt_sb[:], ngt[:])
            idt_sb = pp.tile([128, 128], F32R, tag="idt")
            nc.sync.dma_start(idt_sb[:], idt[:])

            qt = [qkp.tile([128, S], F32R, tag=f"qt{h}", name=f"qt{h}")
                  for h in range(HPC)]
            kt = [qkp.tile([128, S], F32R, tag=f"kt{h}", name=f"kt{h}")
                  for h in range(HPC)]
            vt = [vp.tile([128, HPC * 65], F32R, tag=f"v{s}", name=f"v{s}")
                  for s in range(NS)]

            # ---- phase 1: feature chains (DVE) + V projection ----
            from contextlib import ExitStack
            with (
                tc.tile_pool(name="xt01", bufs=1) as xt01p,
                tc.tile_pool(name="chain", bufs=1) as chp,
            ):
                _es = ExitStack()
                xt27p = _es.enter_context(tc.tile_pool(name="xt27", bufs=1))
                vwp = _es.enter_context(tc.tile_pool(name="vwp", bufs=1))
                vps = _es.enter_context(
                    tc.tile_pool(name="v_ps", bufs=1, space="PSUM"))
                xT = []
                for od in range(2):
                    x_t = xt01p.tile([128, S], F32R, tag=f"xT{od}",
                                     name=f"xT{od}")
                    nc.sync.dma_start(x_t[:], xbT[od * 128:(od + 1) * 128, :])
                    xT.append(x_t)
                vwr = []
                for od in range(8):
                    vw_t = vwp.tile([128, CW], F32R, tag=f"vwr{od}",
                                    name=f"vwr{od}")
                    nc.sync.dma_start(vw_t[:],
                                      vwT[od * 128:(od + 1) * 128, :])
                    vwr.append(vw_t)
                for od in range(2, 8):
                    x_t = xt27p.tile([128, S], F32R, tag=f"xT{od}",
                                     name=f"xT{od}")
                    nc.sync.dma_start(x_t[:], xbT[od * 128:(od + 1) * 128, :])
                    xT.append(x_t)

                args = {}  # (t, path) -> [arg tile per head]

                def chain(t, path, inv_sb, b_sb):
                    """Packed sin/cos args: per head an f16 [128, S] tile,
                    rows 0:64 cos-arg, 64:128 sin-arg (LUT units, wrapped
                    to [-2048, 2048], integers - exact in f16)."""
                    ts2 = chp.tile([128, S], F32, tag="chA", name="ts2",
                                   bufs=2)
                    nc.vector.tensor_scalar(
                        ts2[:], xT[t][:], inv_sb[:, t:t + 1], b_sb[:, t:t + 1],
                        ALU.mult, ALU.add,
                    )
                    kf = chp.tile([128, S], F32, tag="chB", name="kf", bufs=2)
                    nc.vector.tensor_scalar(kf[:], ts2[:], MAGIC, MAGIC,
                                            ALU.add, ALU.subtract)
                    kwS = chp.tile([128, S], F32, tag="chA", name="kwS",
                                   bufs=2)
                    nc.vector.add_range_wrap(kwS[:], kf[:], 0.0, 2048.0,
                                             4096.0)
                    kwC = chp.tile([128, S], F32, tag="chB", name="kwC",
                                   bufs=2)
                    nc.vector.add_range_wrap(kwC[:], kf[:], 1024.0, 2048.0,
                                             4096.0)
                    ats = []
                    for hh in range(2):
                        ag = agp.tile([128, S], F16, tag="arg",
                                      name=f"arg{t}{path}{hh}", bufs=4)
                        rows = slice(hh * 64, hh * 64 + 64)
                        nc.vector.tensor_copy(ag[0:64, :], kwC[rows, :])
                        nc.vector.tensor_copy(ag[64:128, :], kwS[rows, :])
                        ats.append(ag)
                    args[(t, path)] = ats

                def sins(dsts, t, path):
                    for hh in range(2):
                        nc.scalar.activation(dsts[2 * t + hh][:],
                                             args[(t, path)][hh][:],
                                             AF.Sin, scale=C_LUT)

                chain(0, "q", invq_sb, bq_sb)
                chain(0, "k", invk_sb, bk_sb)
                sins(qt, 0, "q")
                sins(kt, 0, "k")
                chain(1, "q", invq_sb, bq_sb)

                # V = x @ vwT, od-major so matmuls run as each od's
                # x-rows land; all 16 accumulators resident (8 banks).
                vpsums = [vps.tile([128, 2 * CW], F32, tag=f"vps{g}",
                                   name=f"vps{g}") for g in range(NS // 2)]
                for od in range(8):
                    for si in range(NS):
                        vslice = vpsums[si // 2][:, (si % 2) * CW:
                                                 (si % 2) * CW + CW]
                        # start=True clears the whole bank: only the even
                        # half may clear; the odd half's first matmul lands
                        # on has_written=0 and overwrites.
                        nc.tensor.matmul(
                            vslice,
                            xT[od][:, si * 128:(si + 1) * 128],
                            vwr[od][:],
                            start=(od == 0 and si % 2 == 0),
                            stop=(od == 7),
                        )
                for si in range(NS):
                    dst = vt[si][:].rearrange(
                        "p (h w) -> p h w", w=65)[:, :, 0:64]
                    vsl = vpsums[si // 2][:, (si % 2) * CW:(si % 2) * CW + CW]
                    vsrc = vsl.rearrange("p (h w) -> p h w", w=64)
                    nc.vector.tensor_copy(dst, vsrc)
                    onescol = vt[si][:].rearrange(
                        "p (h w) -> p h w", w=65)[:, :, 64:65]
                    nc.gpsimd.memset(onescol.bitcast(F32), 1.0)
                chain(1, "k", invk_sb, bk_sb)

                _es.close()  # free xT2-7/vw/psum before phase 2
                # ---- phase 2: attention + out projection ----
                with (
                    tc.tile_pool(name="atp", bufs=1) as ap,
                    tc.tile_pool(name="osb", bufs=1) as op,
                    tc.tile_pool(name="sc_ps", bufs=2, space="PSUM") as scp,
                    tc.tile_pool(name="o_ps", bufs=2, space="PSUM") as opp,
                ):
                    owr = []  # out-proj weights, needed late
                    for hp in range(2):
                        ow_t = op.tile([128, D], F32R, tag=f"owr{hp}",
                                       name=f"owr{hp}")
                        nc.sync.dma_start(ow_t[:], owT[hp * 128:(hp + 1) * 128, :])
                        owr.append(ow_t)
                    pairs = {}  # (qh, hp) -> tile

                    def outproj_od(qh, od, tail=False):
                        qlo = QW * qh
                        pr = scp.tile([128, QW], F32, tag="sc", name="pr")
                        for c2 in range(2):
                            cs = slice(c2 * 512, c2 * 512 + 512)
                            for hp in range(2):
                                nc.tensor.matmul(
                                    pr[:, cs],
                                    owr[hp][:, od * 128:(od + 1) * 128],
                                    pairs[(qh, hp)][:, cs],
                                    start=(hp == 0), stop=(hp == 1),
                                )
                        prsb = op.tile([128, QW], F32, tag="prsb",
                                       name="prsb", bufs=2)
                        if tail and od % 2 == 0:
                            nc.scalar.copy(prsb[:], pr[:])
                        else:
                            nc.vector.tensor_copy(prsb[:], pr[:])
                        nc.sync.dma_start(
                            outT[od * 128:(od + 1) * 128, qlo:qlo + QW],
                            prsb[:],
                        )

                    def head_block(qh, h, fillers=None):
                        qlo = QW * qh
                        kbmax = 8 * qh + 8
                        if (qh, h // 2) not in pairs:
                            pairs[(qh, h // 2)] = op.tile(
                                [128, QW], F32R, tag=f"pairs{h // 2}",
                                name=f"pairs{qh}{h // 2}", bufs=2)
                        o_ps = opp.tile([65, QW], F32, tag="o", name="o_ps")
                        pend = None  # (at, vcol, j0, kb)

                        def flush_av():
                            p_at, p_vcol, p_j0, p_kb = pend
                            for j in range(p_j0, 2):
                                n0 = max(p_vcol, j * 512)
                                n1 = (j + 1) * 512
                                nc.tensor.matmul(
                                    o_ps[:, n0:n1],
                                    vt[p_kb][:, h * 65:(h + 1) * 65],
                                    p_at[:, n0:n1],
                                    start=(p_kb == 0),
                                    stop=(p_kb == 8 * qh + 4 * j + 3),
                                )

                        for kb in range(kbmax):
                            vcol = max(0, 128 * kb - qlo)
                            j0 = vcol // 512
                            dc = 128 * kb - qlo
                            diag = (kb // 8 == qh)
                            sc = scp.tile([128, QW], F32, tag="sc", name="sc")
                            for j in range(j0, 2):
                                n0 = max(vcol, j * 512)
                                n1 = (j + 1) * 512
                                jdiag = diag and (dc // 512 == j)
                                nc.tensor.matmul(
                                    sc[:, n0:n1],
                                    kt[h][:, kb * 128:(kb + 1) * 128],
                                    qt[h][:, qlo + n0:qlo + n1],
                                    start=True, stop=not jdiag,
                                )
                                if jdiag:
                                    nc.tensor.matmul(
                                        sc[:, dc:dc + 128], idt_sb[:],
                                        ngt_sb[:], start=False, stop=True,
                                    )
                            if pend is not None:
                                flush_av()
                            at = ap.tile([128, QW], F32R, tag="at", name="at",
                                         bufs=3)
                            nc.scalar.activation(
                                at[:, vcol:QW], sc[:, vcol:QW], AF.Exp,
                                scale=inv_scale,
                            )
                            pend = (at, vcol, j0, kb)
                            if fillers:
                                fillers.pop(0)()
                        flush_av()
                        # normalize: denom row 64 -> bc -> recip -> mult
                        srow = op.tile([1, QW], F32, tag="srow", name="srow",
                                       bufs=1)
                        nc.vector.tensor_copy(srow[:], o_ps[64:65, :])
                        bc = op.tile([64, QW], F32, tag="bc", name="bc", bufs=1)
                        nc.gpsimd.partition_broadcast(bc[:], srow[:])
                        rec = op.tile([64, QW], F32, tag="rec", name="rec",
                                      bufs=1)
                        nc.vector.reciprocal_approx_fast(out=rec[:],
                                                         in_=bc[:])
                        dstp = pairs[(qh, h // 2)]
                        rows = slice(64 * (h % 2), 64 * (h % 2) + 64)
                        nc.vector.tensor_tensor(dstp[rows, :], o_ps[0:64, :],
                                                rec[:], ALU.mult)

                    head_block(0, 0)
                    head_block(0, 1)
                    head_block(1, 0)
                    head_block(1, 1)
                    sins(qt, 1, "q")
                    sins(kt, 1, "k")
                    head_block(0, 2)
                    head_block(0, 3)
                    op0 = [  # outproj(0) interleaved into the next block
                        (lambda od=od: outproj_od(0, od)) for od in range(8)]
                    head_block(1, 2, fillers=[(lambda: None)] * 4 + op0)
                    head_block(1, 3)
                    for od in range(8):
                        outproj_od(1, od, tail=True)

    nc.compile()
    return nc


def _prep_inputs(x, w_q, b_q, w_k, b_k, v_w, out_w):
    """Build the 8 per-core input maps (host-side sharding)."""
    s_lut = np.float64(LUT) / TWO_PI
    in_maps = []
    ngt = np.where(np.arange(128)[None, :] < np.arange(128)[:, None],
                   np.float32(-1e5), np.float32(0.0)).astype(np.float32)
    idt = np.eye(128, dtype=np.float32)

    wq = w_q.reshape(D)
    bqv = b_q.reshape(D)
    wk = w_k.reshape(D)
    bkv = b_k.reshape(D)

    for c in range(NCORES):
        b = c // 4
        h0 = (c % 4) * HPC
        colbase = h0 * DH
        cols = np.arange(colbase, colbase + CW)
        rest = np.concatenate([np.arange(0, colbase),
                               np.arange(colbase + CW, D)])
        perm = np.concatenate([cols, rest])

        xbT = np.ascontiguousarray(x[b][:, perm].T, dtype=np.float32)
        vwT = np.ascontiguousarray(v_w[cols][:, perm].T, dtype=np.float32)
        owT = np.ascontiguousarray(out_w[:, cols].T, dtype=np.float32)

        def featparams(w, bias):
            inv = s_lut / (1.0 + np.abs(w[cols].astype(np.float64)))
            bb = bias[cols].astype(np.float64) * s_lut
            return (inv.reshape(2, 128).T.astype(np.float32).copy(),
                    bb.reshape(2, 128).T.astype(np.float32).copy())

        iq, bq_ = featparams(wq, bqv)
        ik, bk_ = featparams(wk, bkv)

        in_maps.append(dict(
            xbT=xbT, vwT=vwT, owT=owT,
            invq=iq, bq=bq_, invk=ik, bk=bk_,
            ngt=ngt, idt=idt,
        ))
    return in_maps


def kernel(x, w_q, b_q, w_k, b_k, v_w, out_w, _trace=False):
    x = np.asarray(x, dtype=np.float32)
    w_q = np.asarray(w_q, dtype=np.float32)
    b_q = np.asarray(b_q, dtype=np.float32)
    w_k = np.asarray(w_k, dtype=np.float32)
    b_k = np.asarray(b_k, dtype=np.float32)
    v_w = np.asarray(v_w, dtype=np.float32)
    out_w = np.asarray(out_w, dtype=np.float32)

    if "nc" not in _CACHE:
        _CACHE["nc"] = _build_nc()
    nc = _CACHE["nc"]

    in_maps = _prep_inputs(x, w_q, b_q, w_k, b_k, v_w, out_w)
    res = run_bass_kernel_spmd(
        nc, in_maps, core_ids=list(range(NCORES)), trace=_trace
    )
    out = np.zeros((B, S, D), dtype=np.float32)
    for c in range(NCORES):
        out[c // 4] += res.results[c]["outT"].T
    if _trace:
        kernel._last_result = res
    return out

